# revision 19
# baseline (speedup 1.0000x reference)
"""LocallyConnected2d Bass kernel for 8 Trainium2 NeuronCores.

Problem (hardcoded): x[16,32,64,64] f32, weight[64,64,32,32,3,3] f32,
bias[32,64,64] f32 -> out[16,32,64,64] f32.  stride=1, pad=1, dil=1.

Sharding: outH split across 8 cores (8 rows each).  Per core, per output
row h: 64 w-positions x 3 kernel-rows of matmuls [K<=97,M=32]x[K,N=16]
accumulated in PSUM, K = (kernel-col j)*32 + inC c, plus a 97th
bias-constant row on the ik=0 matmul (x row 96 = 2.0, w row 96 =
2^8*bias): the bias rides the weight stream and the PSUM copy becomes a
pure x0.5 scale.  The kernel is HBM-bandwidth bound on the per-position
weights (360 GB/s cost-model stream, one DMA-engine device); the design
keeps that stream dense and the post-stream tail short:
  - weights fp8 e3m4, scaled 2^8 on host (descale on host - exact).
  - x DMAed once (unreplicated) into partitions 0..31; j=1,2 shifted
    copies made on-chip by DVE (bf16-bitcast 4x mode).  Outer padded
    rows ride fp8, middle rows bf16.  The x row-96 bias constants (2.0)
    ride two tiny DMAs (a DVE memset of the fp8 row would cost ~6us).
  - input DMA order (xb, x8, w0, xc8, w1, xcb, w2, ...) keeps the
    per-DMA HWDGE pipeline (650ns each) ahead of the transfer stream
    (no bubbles) while landing x early enough that the DVE shift-copies
    (tb split in halves) never leave the PE idle - a PE gap resets the
    p-state ramp and halves matmul speed for 3us.
  - weight DMAs taper (rows 0-4 whole, row 5 halves, row 6 quarters,
    row 7 in 4,4,4,2,2-quad pieces) so per-piece matmuls (84ns/quad)
    drain inside each piece's transfer time (103ns/quad) and only the
    last 2 quads' compute remains after the final weight byte.
  - ALL outputs ride SWDGE scatter-add DMAs into four separate dram
    tensors (separate so no false WAW chains through Tile's
    whole-tensor write tracking).  Descriptors are prepared early on
    the idle gpsimd engine; cheap trigger_dma instructions fire each
    transfer as soon as its PSUM copies land - skipping the 625ns HWDGE
    + 650ns DGE-delay a dma_start pays after its wait, which matters
    most for the final row-7 piece on the critical tail.
  - dummy matmuls on a zeroed scratch tile warm the PE p-state ramp.
"""

import numpy as np
import ml_dtypes

B, C, H, W = 16, 32, 64, 64
OC = 32
KH = KW = 3
NCORES = 8
RPC = H // NCORES  # rows per core = 8
NQ = 4  # quad size (PE col groups)
NQW = W // NQ  # 16 quads per row
WSCALE = 2.0**8  # weight scale into fp8e3 range (max 15.08 < 15.5)
XSCALE = 2.0**1  # x scale (fp8e3 chunks; bf16 chunks carry it exactly)
WP = W + 2  # padded width positions per row
KBIAS = 97  # contraction rows incl. the bias-constant row

BF16 = ml_dtypes.bfloat16
F8E3 = ml_dtypes.float8_e3m4

XB = W * B  # x cols per row = 1024
OB = NQW * B  # out cols per row block = 256

# weight DMA pieces per row, in quads
WPIECES = {h: [(0, 16)] for h in range(5)}
WPIECES[5] = [(0, 8), (8, 16)]
WPIECES[6] = [(0, 4), (4, 8), (8, 12), (12, 16)]
WPIECES[7] = [(0, 4), (4, 8), (8, 12), (12, 14), (14, 16)]

# out scatter groups: (name, first row, n rows)
OGROUPS = [("oA", 0, 3), ("oB", 3, 3), ("oC", 6, 1), ("oD", 7, 1)]

_cache = {}


def _build_nc():
    import concourse.bass as bass
    import concourse.tile as tile
    from concourse import bacc, mybir

    nc = bacc.Bacc(
        "TRN2",
        target_bir_lowering=False,
        debug=False,
        num_devices=NCORES,
        num_swdge_queues=4,
    )
    nc.dynamic_dma_scratch_size = 65536  # 4096-desc SWDGE ring
    f32 = mybir.dt.float32
    f16 = mybir.dt.float16
    bf16 = mybir.dt.bfloat16
    f8e3 = mybir.dt.float8e3
    i16 = mybir.dt.int16

    # x, split by row-usage: outer padded rows (hh 0-2, 7-9) as fp8e3,
    # middle rows (hh 3-6) bf16, all scaled by 2.  Partition c holds
    # x[c, hh, wp, b] for non-pad wp=1..64; pads memset on-chip.
    xs8 = nc.dram_tensor("xs8", (32, 6, XB), f8e3, kind="ExternalInput")
    xsb = nc.dram_tensor("xsb", (32, 4, XB), bf16, kind="ExternalInput")
    # bias-row constants (value 2.0) for x partition 96
    xc8 = nc.dram_tensor("xc8", (1, 4, XB), f8e3, kind="ExternalInput")
    xcb = nc.dram_tensor("xcb", (1, 4, XB), bf16, kind="ExternalInput")
    # wt: [8, 97, 64*3*32] f8e3 scaled 2^8; [h, j*32+c, (w*3+ik)*32+o],
    # row 96 = 2^8*bias[o,h,w] at (w*3+0)*32+o (0 elsewhere).
    wt = nc.dram_tensor(
        "wt", (RPC, KBIAS, W * KH * OC), f8e3, kind="ExternalInput"
    )
    # outputs (scatter-add dests, pre-zeroed by the runner):
    # [h, p, q*16+b] per group
    oten = {
        name: nc.dram_tensor(
            name, (nh * 128, OB), f16, kind="ExternalOutput"
        )
        for name, h0, nh in OGROUPS
    }

    with tile.TileContext(nc) as tc:
        with (
            tc.tile_pool(name="xpool", bufs=1) as xpool,
            tc.tile_pool(name="wpool", bufs=1) as wpool,
            tc.tile_pool(name="opool", bufs=1) as opool,
            tc.tile_pool(name="psum", bufs=3, space="PSUM") as ppool,
            tc.tile_pool(name="psum7", bufs=1, space="PSUM") as ppool7,
        ):
            # PE p-state warmup: the cost model runs matmuls at reduced
            # speed until the PE has been continuously busy ~3us, and a
            # PE idle gap restarts the ramp.  Dummy matmuls on a memset
            # scratch tile bridge until the first real matmul (~5.9us).
            NWARM = 54
            wtile = xpool.tile([96, 160], bf16, tag="warm")
            nc.vector.memset(wtile[:], 0.0)
            pwarm = ppool.tile([4 * OC, OB], f32, tag="pt")
            for _ in range(NWARM):
                nc.tensor.matmul(
                    pwarm[0:32, 0:128], wtile[0:96, 0:32],
                    wtile[0:96, 32:160], start=True, stop=True,
                )

            # x tiles [97, rows, WP*B]: fp8 chunk rows hh{0,1,2,7,8,9},
            # bf16 chunk rows hh{3,4,5,6}; partition 96 = 2.0 bias row
            # (DMAed constants; rows 0-3 of each chunk suffice: ik=0
            # reads hh=h<8 only).
            t8 = xpool.tile([KBIAS, 6, WP * B], f8e3, tag="x8")
            tb = xpool.tile([KBIAS, 4, WP * B], bf16, tag="xb")

            def xcopies():
                # j=1,2 column-shifted copies into partitions 32..95.
                # fp8 goes through a bf16 bitcast (paired elements) so
                # DVE keeps 4x 2-byte perf mode.  tb is split in
                # row-halves so row 1's matmuls (which need tb row 0)
                # don't wait for the full copy.
                for j in (1, 2):
                    dst = t8[32 * j : 32 * (j + 1), :, 0:XB].bitcast(bf16)
                    src = t8[0:32, :, j * B : j * B + XB].bitcast(bf16)
                    nc.vector.tensor_copy(dst, src)
                for r0, r1 in ((0, 2), (2, 4)):
                    for j in (1, 2):
                        dst = tb[32 * j : 32 * (j + 1), r0:r1, 0:XB]
                        src = tb[0:32, r0:r1, j * B : j * B + XB]
                        nc.vector.tensor_copy(dst, src)

            def xslice(hh, w, k):
                if hh < 3:
                    return t8[0:k, hh, w * B : (w + 1) * B]
                if hh < 7:
                    return tb[0:k, hh - 3, w * B : (w + 1) * B]
                return t8[0:k, hh - 4, w * B : (w + 1) * B]

            wtiles = {h: [] for h in range(RPC)}

            def load_w(h):
                for pi, (q0, q1) in enumerate(WPIECES[h]):
                    c0, c1 = q0 * NQ * KH * OC, q1 * NQ * KH * OC
                    t = wpool.tile([KBIAS, c1 - c0], f8e3, tag=f"w{h}_{pi}")
                    nc.sync.dma_start(t[:], wt[h, :, c0:c1])
                    wtiles[h].append((q0 * NQ, q1 * NQ, t))

            # input DMA order: x8, xb, w0, xc8, w1, xcb, w2, ... so the
            # 650ns-per-DMA HWDGE pipeline stays ahead of the stream and
            # the t8 shift-copies (needed by row 0) run first on DVE.
            nc.sync.dma_start(t8[0:32, :, B : B + XB], xs8[:, :])
            nc.sync.dma_start(tb[0:32, :, B : B + XB], xsb[:, :])
            for t in (t8, tb):
                nc.vector.memset(t[0:32, :, 0:B], 0.0)
                nc.vector.memset(t[0:32, :, B + XB : WP * B], 0.0)
            for h in range(RPC):
                load_w(h)
                if h == 0:
                    nc.sync.dma_start(t8[96:97, 0:4, 0:XB], xc8[:, :, :])
                if h == 1:
                    nc.sync.dma_start(tb[96:97, 0:4, 0:XB], xcb[:, :, :])

            def wslice(h, w, ik, k):
                for w0, w1, t in wtiles[h]:
                    if w0 <= w < w1:
                        return t[0:k, ((w - w0) * 3 + ik) * 32 :][:, 0:32]
                raise AssertionError

            xcopies()

            def mm_quads(h, pt, q0, q1, pq0):
                for q in range(q0, q1):
                    for g in range(NQ):
                        w = q * NQ + g
                        for ik in range(KH):
                            k = KBIAS if ik == 0 else 96
                            nc.tensor.matmul(
                                pt[
                                    32 * g : 32 * (g + 1),
                                    (q - pq0) * B : (q - pq0 + 1) * B,
                                ],
                                wslice(h, w, ik, k),
                                xslice(h + ik, w, k),
                                start=(ik == 0),
                                stop=(ik == 2),
                                tile_position=(0, 32 * g),
                            )

            # single out tile [128, 8*256] f16; row h at cols h*256.
            ot = opool.tile([4 * OC, RPC * OB], f16, tag="out")

            def copy_piece(eng, h, q0, q1, pt, pq0):
                # out = psum * 0.5 (2^9-scaled sums incl bias -> 2^8)
                dst = ot[:, h * OB + q0 * B : h * OB + q1 * B]
                src = pt[:, (q0 - pq0) * B : (q1 - pq0) * B]
                if eng == "act":
                    nc.scalar.activation(
                        dst, src, mybir.ActivationFunctionType.Copy,
                        scale=0.5,
                    )
                else:
                    nc.vector.tensor_scalar(
                        dst, src, 0.5, None, mybir.AluOpType.mult
                    )

            # scratch used as a WAW gate: the early out-triggers "write"
            # it (signals_writable) and the memset below (emitted after
            # copy6) writes it first, so those triggers can't fire and
            # preempt the DMA engines before the input stream drains.
            scr = xpool.tile([1, 8], f32, tag="scr")

            for h in range(RPC):
                if h < 7:
                    pt = ppool.tile([4 * OC, OB], f32, tag="pt")
                    for q0, q1 in WPIECES[h]:
                        mm_quads(h, pt, q0, q1, 0)
                    copy_piece("dve", h, 0, NQW, pt, 0)
                    if h == 6:
                        nc.vector.memset(scr[:], 0.0)
                else:
                    for pi, (q0, q1) in enumerate(WPIECES[h]):
                        pt = ppool7.tile(
                            [4 * OC, (q1 - q0) * B], f32, tag=f"p7{pi}"
                        )
                        mm_quads(h, pt, q0, q1, q0)
                        copy_piece(
                            "act" if pi == 2 else "dve", h, q0, q1, pt, q0
                        )

            # outputs: prepare all scatter descriptors (idle gpsimd),
            # then trigger each group as its copies land.
            idxt = xpool.tile([16, 64], i16, tag="idx")
            io = 0
            regions = []
            for qn, (name, h0, nh) in enumerate(OGROUPS):
                n16 = nh * 8
                nc.gpsimd.iota(
                    idxt[:, io : io + n16], [[16, n16]], base=0,
                    channel_multiplier=1,
                )
                regions.append((qn, name, h0, nh, io))
                io += n16
            for qn, name, h0, nh, io in regions:
                a = ot[:, h0 * OB : (h0 + nh) * OB]
                src = bass.AP(
                    a.tensor, a.offset,
                    [list(a.ap[0]), [OB, nh], [1, OB]],
                )
                d = oten[name][:, :]
                dst = bass.AP(d.tensor, d.offset, [[OB, nh * 128], [1, OB]])
                sem = nc.alloc_semaphore(f"sca{qn}")
                nc.gpsimd.dma_scatter_add(
                    dst, src, idxt[:, io : io + nh * 8], nh * 128,
                    nh * 128, OB, prepare_only=True, sem=sem, queue_num=qn,
                )
            # Pool-FIFO gate: this copy reads scr (written after copy6),
            # so the in-order Pool sequencer can't reach the triggers -
            # and preempt the DMA engines mid input-stream - before the
            # input stream has drained.
            nc.gpsimd.tensor_copy(scr[0:1, 4:8], scr[0:1, 0:4])
            for qn, name, h0, nh, io in regions:
                nc.gpsimd.trigger_dma(count=None, queue_num=qn)

    # Tile puts each SWDGE prep on a DMASW lane and gates the closing
    # barrier on that lane's semaphore, but the DMA-completion sem baked
    # into the descriptor is the user-passed `sem=`.  Re-point
    # OnUpdate[0] (the descriptor sem) at the assigned lane sem so the
    # completion actually ticks the lane.
    fn = nc.m.functions[0]
    lane_sems = {}
    insts = []
    for bb in fn.blocks:
        for i in bb.instructions:
            insts.append(i)
            si = i.sync_info
            if si:
                for w in si.on_wait:
                    if w.ant_name and w.ant_name.startswith("DMASW"):
                        lane_sems[int(w.ant_name[5:].split("_")[0])] = (
                            w.id, w.ant_name,
                        )
    for i in insts:
        if (
            type(i).__name__ == "InstDMAScatterAddAnt"
            and getattr(i, "gen_mode", 0) == 1
        ):
            lane = i.bass_scheduled_proc - 11
            assert lane in lane_sems, (lane, lane_sems)
            u0 = i.sync_info.on_update[0]
            u0.id, u0.ant_name = lane_sems[lane]
    nc.compile()
    return nc


def _prep_inputs(x, weight, bias):
    """Host-side shard + layout prep.  Returns list of 8 per-core dicts."""
    # padded x, transposed to [c, hh, wp, b], scaled by 2 (f32 master)
    xp = np.zeros((C, H + 2, W + 2, B), dtype=np.float32)
    xp[:, 1 : H + 1, 1 : W + 1, :] = np.ascontiguousarray(
        x.transpose(1, 2, 3, 0) * np.float32(XSCALE)
    )

    # weight -> [h, j, c, w, ik, o] scaled into fp8e3 range, plus the
    # bias row: wrow96[h, (w*3+0)*32+o] = 2^8 * bias[o, h, w]
    wtr = np.ascontiguousarray(
        weight.transpose(0, 5, 3, 1, 4, 2) * np.float32(WSCALE)
    ).astype(F8E3)
    wtr = wtr.reshape(H, 96, W * KH * OC)
    brow = np.zeros((H, 1, W, KH, OC), dtype=np.float32)
    brow[:, 0, :, 0, :] = bias.astype(np.float32).transpose(1, 2, 0) * (
        np.float32(WSCALE)
    )
    brow = brow.reshape(H, 1, W * KH * OC).astype(F8E3)
    wall = np.concatenate([wtr, brow], axis=1)  # [H, 97, 6144]

    xc8 = np.full((1, 4, XB), 2.0, dtype=F8E3)
    xcb = np.full((1, 4, XB), 2.0, dtype=BF16)

    in_maps = []
    for i in range(NCORES):
        h0 = i * RPC
        xcore = xp[:, h0 : h0 + RPC + 2, :, :]
        xn = xcore[:, :, 1 : W + 1]  # [32, 10, 64, 16] non-pad cols
        x8 = np.concatenate([xn[:, 0:3], xn[:, 7:10]], axis=1)
        in_maps.append(
            {
                "xs8": np.ascontiguousarray(
                    x8.astype(F8E3).reshape(32, 6, XB)
                ),
                "xsb": np.ascontiguousarray(
                    xn[:, 3:7].astype(BF16).reshape(32, 4, XB)
                ),
                "xc8": xc8,
                "xcb": xcb,
                "wt": np.ascontiguousarray(wall[h0 : h0 + RPC]),
            }
        )
    return in_maps


def _run(in_maps, trace=False, tmpdir=None):
    from concourse.bass_utils import run_bass_kernel_spmd

    if "nc" not in _cache:
        _cache["nc"] = _build_nc()
    return run_bass_kernel_spmd(
        _cache["nc"], in_maps, list(range(NCORES)), trace=trace, tmpdir=tmpdir
    )


def _assemble(results):
    out = np.empty((B, OC, H, W), dtype=np.float32)
    inv = np.float32(1.0 / WSCALE)

    def put(core_i, h, block):
        # block: [128, 256] f32 for output row h of this core:
        # [g*32+o, q*16+b] -> out[b, o, h0+h, q*4+g]
        blk = block.reshape(NQ, OC, NQW, B) * inv
        out[:, :, core_i * RPC + h, :] = blk.transpose(3, 1, 2, 0).reshape(
            B, OC, W
        )

    for i in range(NCORES):
        for name, h0, nh in OGROUPS:
            r = results[i][name].astype(np.float32).reshape(nh, 128, OB)
            for j in range(nh):
                put(i, h0 + j, r[j])
    return out


def kernel(x, weight, bias):
    x = np.asarray(x)
    weight = np.asarray(weight)
    bias = np.asarray(bias)
    in_maps = _prep_inputs(x, weight, bias)
    results = _run(in_maps).results
    return _assemble(results)
